# revision 5
# baseline (speedup 1.0000x reference)
"""Trainium2 Bass kernel for nn_NCC2D (v3: DRAM-round-trip row shifts).

B=32 images of 512x512 fp32; 8 cores x 4 images (pure data parallel).

Cost model measured on this axon-trn2 env (per instruction, nearly flat
in size up to ~5MB/operand):
  DVE tensor_tensor ~35-40us, stt ~1.3x, recip ~62us, ttr ~44us
  partition-shifted SBUF->SBUF DMA: 242us/5.2MB  (the v2 bottleneck)
  aligned SBUF<->DRAM DMA: ~36-52us/5.2MB; HBM cast load ~60us/2MB read
  per-op dispatch floor ~20us; ACT ~135us flat; Pool ~2.5x DVE

v3 design:
  - p-major row layout: image row r lives at (partition p=r//4, b=r%4),
    so row shifts become byte offsets in DRAM: the row tree stores the
    col-filtered maps to a DRAM scratch (aligned, fast) and reloads at
    row offsets +-1 / +-3 (also aligned) instead of partition-shifted
    SBUF DMAs. Zero guard rows in the scratch give exact image edges.
  - 2 half-passes of 2 images; hp-major slot layout [hp*10 + map*2+img]
    with maps (I,J,IJ,II,JJ); 520-wide rows with 4-col zero margins
    (memset once; never rewritten) give exact column edges.
  - products: 2 DVE ops (IJ; II+JJ as one fat square).
  - col 9-tap: 4 DVE ops (3+3x3 tree, free-dim AP shifts).
  - row 9-tap: 2 stores + 4 shifted loads (DMA) + 4 DVE adds.
  - cc once per rep over all 4 images: fused stt folds the /81 terms,
    tensor_tensor_reduce folds cc*recip and the global sum.
"""

import numpy as np

H = W = 512
B = 32
NCORES = 8
IMGS = B // NCORES      # 4 images per core
IPP = 2                 # images per half-pass
NHP = IMGS // IPP       # 2 half-passes
NACC = 1                # accumulator columns
SEG = 520               # 4 + 512 + 4 zero col margins
NS = 10                 # slots per half-pass (5 maps x 2 images)
GR = 528                # guard-padded rows per map block in scratch


def build_bass(reps=1, skips=()):
    from contextlib import ExitStack

    import concourse.tile as tile
    from concourse import bacc, mybir

    f32 = mybir.dt.float32
    f16 = mybir.dt.float16
    Alu = mybir.AluOpType

    nc = bacc.Bacc(
        "TRN2",
        target_bir_lowering=False,
        debug=False,
        num_devices=NCORES,
    )
    yt = nc.dram_tensor("y_true", [IMGS, H, W], f32, kind="ExternalInput").ap()
    yp = nc.dram_tensor("y_pred", [IMGS, H, W], f32, kind="ExternalInput").ap()
    out_d = nc.dram_tensor("partial", [128, 1], f32, kind="ExternalOutput").ap()
    scr_y = nc.dram_tensor("scr_y", [NS * GR, SEG], f16, kind="Internal").ap()
    scr_z = nc.dram_tensor("scr_z", [NS * GR, SEG], f16, kind="Internal").ap()

    with tile.TileContext(nc) as tc, ExitStack() as ctx:
        pool = ctx.enter_context(tc.tile_pool(name="p", bufs=1))
        MM = pool.tile([128, 2 * NS, 4, SEG], f16)   # 81.3KB/part
        SS = pool.tile([128, 64 + 2 * NS * 4 * SEG], f16)  # 81.4KB/part
        T4 = pool.tile([128, NS, 4, SEG], f16)
        acc = pool.tile([128, 1], f32)

        # one-time init: zero margins everywhere + guard rows in scratch
        nc.vector.memset(MM[:], 0.0)
        nc.vector.memset(SS[:], 0.0)
        nc.vector.memset(T4[:], 0.0)
        HNS = NS * 4 * SEG
        S1 = SS[:, 64 : 64 + HNS].rearrange(
            "p (s b c) -> p s b c", s=NS, c=SEG
        )
        S2 = SS[:, 64 + HNS : 64 + 2 * HNS].rearrange(
            "p (s b c) -> p s b c", s=NS, c=SEG
        )
        SSf32 = SS[:].bitcast(mybir.dt.float32)  # [p, 32 + HNS]
        sy = scr_y.rearrange("(mi r) c -> mi r c", mi=NS)
        sz = scr_z.rearrange("(mi r) c -> mi r c", mi=NS)
        T4f = T4.rearrange("p s b c -> p (s b c)")
        for sv in (sy, sz):
            nc.sync.dma_start(
                sv[:, 0:8, :].rearrange("mi r c -> mi (r c)"),
                T4f[0:NS, 0 : 8 * SEG],
            )
            nc.sync.dma_start(
                sv[:, GR - 8 : GR, :].rearrange("mi r c -> mi (r c)"),
                T4f[0:NS, 0 : 8 * SEG],
            )

        # flat-slot views [p, slots, 2080]
        MMe = MM.rearrange("p s b c -> p s (b c)")
        S1e = S1.rearrange("p s b c -> p s (b c)")
        S2e = S2.rearrange("p s b c -> p s (b c)")
        T4e = T4.rearrange("p s b c -> p s (b c)")
        S1f = S1.rearrange("p s b c -> p (s b c)")
        S2f = S2.rearrange("p s b c -> p (s b c)")
        T4flat = T4.rearrange("p s b c -> p (s b c)")
        # 4-image views of MM: [p, hp, slot, 2080]
        MMh = MM.rearrange("p (h s) b c -> p h s (b c)", h=2)

        def hsplit(view2):
            # [p, 2k, 2080] -> [p, 2, k, 2080]
            return view2.rearrange("p (h s) e -> p h s e", h=2)

        def shift_view(sv, off):
            # rows off..off+512 of each map block as [p, mi, (b c)]
            return sv[:, 8 + off : 8 + off + H, :].rearrange(
                "mi (p b) c -> p mi (b c)", p=128
            )

        for rep in range(reps):
            for hp in range(NHP):
                base = hp * NS
                M = MM[:, base : base + NS]            # [p,10,4,520]
                Mf = M.rearrange("p s b c -> p (s b c)")
                Mh = M.rearrange("p s b c -> p s (b c)")
                # ---- input loads (cast f32->f16), p-major rows ------
                if "loads" not in skips:
                    for i in range(IPP):
                        img = hp * IPP + i
                        src_t = yt[img].rearrange("(p b) c -> p b c", p=128)
                        src_p = yp[img].rearrange("(p b) c -> p b c", p=128)
                        nc.gpsimd.dma_start(M[:, 0 + i, :, 4:516], src_t)
                        nc.gpsimd.dma_start(M[:, 2 + i, :, 4:516], src_p)
                # ---- products: IJ, then II+JJ as one fat square -----
                nc.vector.tensor_tensor(
                    Mh[:, 4:6], Mh[:, 0:2], Mh[:, 2:4], op=Alu.mult
                )
                nc.vector.tensor_tensor(
                    Mh[:, 6:10], Mh[:, 0:4], Mh[:, 0:4], op=Alu.mult
                )
                # ---- col 9-tap via f32 scan + shifted subtract ------
                # s = running sum (f32, exact-ish); box[c] = s[c+4]-s[c-5]
                nc.vector.tensor_tensor_scan(
                    SSf32[:, 32 : 32 + NS * 4 * SEG], Mf, Mf,
                    0.0, op0=Alu.add, op1=Alu.bypass,
                )
                Wp = SSf32[:, 32 : 32 + NS * 4 * SEG].rearrange(
                    "p (s b c) -> p s b c", s=NS, c=SEG
                )
                Wm = SSf32[:, 31 : 31 + NS * 4 * SEG].rearrange(
                    "p (s b c) -> p s b c", s=NS, c=SEG
                )
                nc.vector.tensor_tensor(
                    M[:, :, :, 4:516], Wp[:, :, :, 8:520], Wm[:, :, :, 0:512],
                    op=Alu.subtract,
                )
                # ---- row 9-tap via DRAM round trips -----------------
                if "rowdma" not in skips:
                    nc.sync.dma_start(shift_view(sy, 0), Mh)
                if "rowdma" not in skips:
                    nc.sync.dma_start(S1e[:, :, :], shift_view(sy, 1))
                if "rowdma" not in skips:
                    nc.sync.dma_start(S2e[:, :, :], shift_view(sy, -1))
                nc.vector.tensor_tensor(T4flat, Mf, S1f, op=Alu.add)
                nc.vector.tensor_tensor(Mf, T4flat, S2f, op=Alu.add)
                if "rowdma" not in skips:
                    nc.sync.dma_start(shift_view(sz, 0), Mh)
                if "rowdma" not in skips:
                    nc.sync.dma_start(S1e[:, :, :], shift_view(sz, 3))
                if "rowdma" not in skips:
                    nc.sync.dma_start(S2e[:, :, :], shift_view(sz, -3))
                nc.vector.tensor_tensor(T4flat, Mf, S1f, op=Alu.add)
                nc.vector.tensor_tensor(Mf, T4flat, S2f, op=Alu.add)
            if "cc" in skips:
                nc.vector.memset(acc[:], 0.0)
            if "cc" not in skips:
                # ---- cc over all 4 images ------------------------------
                # MM slots per hp: 0,1=a 2,3=b 4,5=E 6,7=C 8,9=D (box maps)
                ab4 = MMh[:, :, 0:4]      # [p,2,4,2080]
                a4 = MMh[:, :, 0:2]
                b4 = MMh[:, :, 2:4]
                E4 = MMh[:, :, 4:6]
                CD4 = MMh[:, :, 6:10]
                sq8 = hsplit(T4e[:, 0:8])         # [p,2,4,2080]
                ivjv = hsplit(S2e[:, 0:8])        # [p,2,4,2080]
                den = hsplit(T4e[:, 0:4])         # [p,2,2,2080]
                t0 = hsplit(S1e[:, 0:4])
                cross = hsplit(T4e[:, 4:8])
                num = hsplit(S1e[:, 4:8])
                # t1,t2 = a^2,b^2 (all 4 imgs, one op)
                nc.vector.tensor_tensor(sq8, ab4, ab4, op=Alu.mult)
                nc.vector.tensor_tensor(t0, a4, b4, op=Alu.mult)
                nc.vector.scalar_tensor_tensor(
                    ivjv, sq8, -1.0 / 81.0, CD4, op0=Alu.mult, op1=Alu.add
                )
                nc.vector.scalar_tensor_tensor(
                    cross, t0, -1.0 / 81.0, E4, op0=Alu.mult, op1=Alu.add
                )
                nc.vector.tensor_tensor(
                    den, ivjv[:, :, 0:2], ivjv[:, :, 2:4], op=Alu.mult
                )
                nc.vector.tensor_tensor(num, cross, cross, op=Alu.mult)
                # recip (SBUF f16, valid windows only: den margins are 0) and
                # fused cc = num*recip + global sum, each ONE op over 4 imgs
                with nc.allow_low_precision(reason="f16 recip ok at 2e-2 budget"):
                    nc.vector.reciprocal(
                        S2[:, 0:4, :, 4:516], T4[:, 0:4, :, 4:516]
                    )
                # cc = num * recip into margin-clean S2 slots 4..7, then a flat
                # full-width reduce (margins are exactly zero, so they add 0)
                nc.vector.scalar_tensor_tensor(
                    S2[:, 4:8, :, 4:516],
                    S1[:, 4:8, :, 4:516],
                    0.0,
                    S2[:, 0:4, :, 4:516],
                    op0=Alu.bypass,
                    op1=Alu.mult,
                    accum_out=acc[:, 0:1],
                )
        nc.sync.dma_start(out_d[:], acc[:])

    nc.compile()
    return nc


_CACHED = {}


def kernel(y_true: np.ndarray, y_pred: np.ndarray) -> np.ndarray:
    from concourse import bass_utils

    if "nc" not in _CACHED:
        _CACHED["nc"] = build_bass()
    nc = _CACHED["nc"]

    yt = np.ascontiguousarray(y_true.reshape(B, H, W), dtype=np.float32)
    yp = np.ascontiguousarray(y_pred.reshape(B, H, W), dtype=np.float32)
    in_maps = []
    for c in range(NCORES):
        in_maps.append(
            {
                "y_true": yt[IMGS * c : IMGS * (c + 1)],
                "y_pred": yp[IMGS * c : IMGS * (c + 1)],
            }
        )
    res = bass_utils.run_bass_kernel_spmd(
        nc, in_maps, core_ids=list(range(NCORES))
    )
    total = 0.0
    for c in range(NCORES):
        total += float(res.results[c]["partial"].astype(np.float64).sum())
    mean = total / float(B * H * W)
    return np.float32(-mean)
